# revision 4
# baseline (speedup 1.0000x reference)
"""BitLinear (ternary-weight + int8-activation quantized linear) TRN2 kernel.

Reference computation (per full input):
    scale = mean(|w|)                               # per-tensor weight scale
    w_q   = scale * clip(round(w / (scale+1e-8)), -1, 1)
    gamma = max(max|x| over last dim, 1e-8)         # per-row activation absmax
    x_q   = clip(round(x * 127/gamma), -128, 127) * gamma/127
    out   = x_q @ w_q.T + bias

Key numerical identity used here: with n = round(clip(127*x/gamma)) (integers in
[-127,127], exactly representable in bf16) and t = clip(round(w/scale'),-1,1)
(ternary, exact in bf16), the matmul n @ t.T accumulates integers < 2^24 in fp32
PSUM, so the bf16 tensor-engine path is numerically exact; the final result is
(gamma[s]*scale/127) * (n @ t.T)[s,o] + bias[o].

Sharding: pure data-parallel over the batch dim of x (8 batches -> 8 cores),
weight/bias replicated per core.  No collectives needed.
"""

import os
import sys
from contextlib import ExitStack

import numpy as np

for _p in ("/opt/trn_rl_repo", os.path.expanduser("~/.axon_site/_ro/trn_rl_repo")):
    if os.path.isdir(_p) and _p not in sys.path:
        sys.path.append(_p)

import concourse.bass as bass
import concourse.tile as tile
from concourse import bacc, mybir

P = 128
MMF = 512  # matmul free-dim block (one PSUM bank of fp32)
F32 = mybir.dt.float32
BF16 = mybir.dt.bfloat16
ALU = mybir.AluOpType
AFT = mybir.ActivationFunctionType
AXL = mybir.AxisListType
C_RND = 12582912.0  # 1.5 * 2**23: adding+subtracting rounds f32 to nearest int

# Full problem dims
B_FULL, S_FULL, D_FULL, O_FULL = 8, 2048, 2048, 2048
N_CORES = 8


def emit_bitlinear(ctx, tc, x_ap, w_ap, b_ap, out_ap, S, D, O, with_bias):
    nc = tc.nc
    SB, OT, DB, ON = S // P, O // P, D // P, O // MMF
    SCH = 512  # s-chunk for the activation transpose round-trip
    NSC, SPC = S // SCH, SCH // P

    dram = ctx.enter_context(tc.tile_pool(name="dram", bufs=1, space="DRAM"))
    t_dram = dram.tile([O, D], BF16, name="t_dram")
    n_dram = [dram.tile([SCH, D], BF16, tag=f"nd{i}", name=f"n_dram{i}")
              for i in range(NSC)]

    stats = ctx.enter_context(tc.tile_pool(name="stats", bufs=1))
    psum = ctx.enter_context(tc.tile_pool(name="psum", bufs=8, space="PSUM"))

    ones_row = stats.tile([1, P], F32)
    nc.any.memset(ones_row[:], 1.0)

    # ---------------- weight phase: scale + ternarize -> t_dram ----------------
    s1 = ExitStack()
    wpool = s1.enter_context(tc.tile_pool(name="wpool", bufs=OT))
    tern = s1.enter_context(tc.tile_pool(name="tern", bufs=3))

    wsum = stats.tile([P, OT], F32)
    w_tiles = []
    for ot in range(OT):
        wt = wpool.tile([P, D], F32, tag="w", name=f"wt{ot}")
        nc.sync.dma_start(wt[:], w_ap[ot * P:(ot + 1) * P, :])
        w_tiles.append(wt)
        nc.vector.tensor_reduce(
            wsum[:, ot:ot + 1], wt[:], axis=AXL.X, op=ALU.add,
            apply_absolute_value=True,
        )
    wsum1 = stats.tile([P, 1], F32)
    nc.vector.tensor_reduce(wsum1[:], wsum[:], axis=AXL.X, op=ALU.add)
    stot = stats.tile([1, 1], F32)
    nc.gpsimd.tensor_reduce(stot[:], wsum1[:], axis=AXL.C, op=ALU.add)

    # scale = stot/(O*D); inv_w = 1/(scale + 1e-8)
    scale_eps = stats.tile([1, 1], F32)
    nc.vector.tensor_scalar(
        scale_eps[:], stot[:], 1.0 / (O * D), 1e-8, op0=ALU.mult, op1=ALU.add)
    inv_w = stats.tile([1, 1], F32)
    nc.vector.reciprocal(inv_w[:], scale_eps[:])
    scale_true = stats.tile([1, 1], F32)
    nc.vector.tensor_scalar(
        scale_true[:], stot[:], 1.0 / (O * D), None, op0=ALU.mult)

    # broadcast scalars across partitions via a K=1 PE matmul
    def bcast_pp(src11, name):
        pb = psum.tile([P, 1], F32, tag="ps")
        nc.tensor.matmul(pb[:], lhsT=ones_row[:], rhs=src11[:], start=True,
                         stop=True)
        dst = stats.tile([P, 1], F32, tag=name)
        nc.scalar.activation(dst[:], pb[:], AFT.Copy)
        return dst

    inv_w_pp = bcast_pp(inv_w, "invwpp")
    wscale_pp = bcast_pp(scale_true, "wscpp")

    # ternarize: t = clip(round(w*inv_w), -1, 1) in bf16, stored to t_dram
    for ot in range(OT):
        wt = w_tiles[ot]
        nc.vector.tensor_scalar(
            wt[:], wt[:], inv_w_pp[:], C_RND, op0=ALU.mult, op1=ALU.add)
        nc.vector.tensor_scalar(
            wt[:], wt[:], C_RND, 1.0, op0=ALU.subtract, op1=ALU.min)
        tt = tern.tile([P, D], BF16, tag="tern")
        nc.vector.tensor_scalar(tt[:], wt[:], -1.0, None, op0=ALU.max)
        nc.sync.dma_start(t_dram[ot * P:(ot + 1) * P, :], tt[:])
    s1.close()

    # transposed re-load: tT[db] holds t.T[d-block, all o]
    tTp = ctx.enter_context(tc.tile_pool(name="tTp", bufs=DB))
    tT = []
    for db in range(DB):
        t4 = tTp.tile([P, O], BF16, tag="tT", name=f"tT{db}")
        nc.sync.dma_start(t4[:], t_dram[:, db * P:(db + 1) * P], transpose=True)
        tT.append(t4)

    # ---------------- activation phase: quantize -> n_dram ----------------
    s2 = ExitStack()
    xpool = s2.enter_context(tc.tile_pool(name="xpool", bufs=3))
    npool = s2.enter_context(tc.tile_pool(name="npool", bufs=3))
    small = s2.enter_context(tc.tile_pool(name="small", bufs=4))

    comb = stats.tile([P, SB], F32)  # gamma*scale/127, per s-row
    for sb in range(SB):
        xt = xpool.tile([P, D], F32, tag="x")
        nc.sync.dma_start(xt[:], x_ap[sb * P:(sb + 1) * P, :])
        g1 = small.tile([P, 1], F32, tag="g1")
        nc.vector.tensor_reduce(
            g1[:], xt[:], axis=AXL.X, op=ALU.max, apply_absolute_value=True)
        nc.vector.tensor_scalar(g1[:], g1[:], 1e-8, None, op0=ALU.max)
        inv1 = small.tile([P, 1], F32, tag="inv1")
        nc.vector.reciprocal(inv1[:], g1[:])
        nc.vector.tensor_scalar(
            comb[:, sb:sb + 1], g1[:], wscale_pp[:], 1.0 / 127.0,
            op0=ALU.mult, op1=ALU.mult)
        nc.vector.tensor_scalar(
            xt[:], xt[:], inv1[:], 127.0, op0=ALU.mult, op1=ALU.mult)
        nn = npool.tile([P, D], BF16, tag="n")
        nc.vector.tensor_scalar(
            nn[:], xt[:], C_RND, C_RND, op0=ALU.add, op1=ALU.subtract)
        sc, so = sb // SPC, sb % SPC
        nc.sync.dma_start(n_dram[sc][so * P:(so + 1) * P, :], nn[:])
    s2.close()

    # transposed re-load: nT[db][sc] holds n.T[d-block, s-chunk]
    nTp = ctx.enter_context(tc.tile_pool(name="nTp", bufs=DB * NSC))
    nT = [[None] * NSC for _ in range(DB)]
    for sc in range(NSC):
        for db in range(DB):
            t4 = nTp.tile([P, SCH], BF16, tag="nT", name=f"nT{db}_{sc}")
            nc.sync.dma_start(
                t4[:], n_dram[sc][:, db * P:(db + 1) * P], transpose=True)
            nT[db][sc] = t4

    # optional bias broadcast [P, O]
    bias_bc = None
    if with_bias:
        brow = stats.tile([1, O], F32)
        nc.sync.dma_start(brow[:], b_ap[:].rearrange("(a b) -> a b", a=1))
        bias_bc = stats.tile([P, O], F32)
        for ob in range(ON):
            pbb = psum.tile([P, MMF], F32, tag="ps")
            nc.tensor.matmul(pbb[:], lhsT=ones_row[:],
                             rhs=brow[:, ob * MMF:(ob + 1) * MMF],
                             start=True, stop=True)
            nc.scalar.activation(
                bias_bc[:, ob * MMF:(ob + 1) * MMF], pbb[:], AFT.Copy)

    # ---------------- matmul + drain ----------------
    outp = ctx.enter_context(tc.tile_pool(name="outp", bufs=3))
    for sb in range(SB):
        sc, so = sb // SPC, sb % SPC
        ps = [psum.tile([P, MMF], F32, tag="ps", name=f"ps{sb}_{i}") for i in range(ON)]
        for db in range(DB):
            lhs = nT[db][sc][:, so * P:(so + 1) * P]
            for ob in range(ON):
                nc.tensor.matmul(
                    ps[ob][:], lhsT=lhs,
                    rhs=tT[db][:, ob * MMF:(ob + 1) * MMF],
                    start=(db == 0), stop=(db == DB - 1))
        osb = outp.tile([P, O], F32, tag="out")
        for ob in range(ON):
            nc.scalar.activation(
                osb[:, ob * MMF:(ob + 1) * MMF], ps[ob][:], AFT.Copy,
                scale=comb[:, sb:sb + 1])
        if with_bias:
            nc.vector.tensor_tensor(osb[:], osb[:], bias_bc[:], op=ALU.add)
        nc.sync.dma_start(out_ap[sb * P:(sb + 1) * P, :], osb[:])


def build_module(S=S_FULL, D=D_FULL, O=O_FULL, with_bias=False):
    nc = bacc.Bacc("TRN2", target_bir_lowering=False, debug=False,
                   enable_asserts=False)
    x_t = nc.dram_tensor("x", [S, D], F32, kind="ExternalInput")
    w_t = nc.dram_tensor("w", [O, D], F32, kind="ExternalInput")
    b_t = nc.dram_tensor("bias", [O], F32, kind="ExternalInput")
    out_t = nc.dram_tensor("out", [S, O], F32, kind="ExternalOutput")
    with tile.TileContext(nc) as tc:
        with ExitStack() as ctx:
            emit_bitlinear(ctx, tc, x_t.ap(), w_t.ap(), b_t.ap(), out_t.ap(),
                           S, D, O, with_bias)
    nc.compile()
    return nc


_CACHE = {}


def _get_module(with_bias):
    key = with_bias
    if key not in _CACHE:
        _CACHE[key] = build_module(with_bias=with_bias)
    return _CACHE[key]


def kernel(x, weight, bias):
    from concourse.bass_utils import run_bass_kernel_spmd

    x = np.asarray(x, dtype=np.float32)
    weight = np.ascontiguousarray(np.asarray(weight, dtype=np.float32))
    bias = np.ascontiguousarray(np.asarray(bias, dtype=np.float32))
    assert x.shape == (B_FULL, S_FULL, D_FULL), x.shape
    with_bias = bool(np.any(bias != 0.0))
    nc = _get_module(with_bias)
    in_maps = [
        {"x": np.ascontiguousarray(x[c]), "w": weight, "bias": bias}
        for c in range(N_CORES)
    ]
    res = run_bass_kernel_spmd(nc, in_maps, list(range(N_CORES)))
    out = np.stack([res.results[c]["out"] for c in range(N_CORES)], axis=0)
    return out.astype(np.float32)
